# revision 26
# baseline (speedup 1.0000x reference)
"""AnchorDML Trainium2 kernel: 8-core SPMD, data-parallel over x rows with
sharded anchor encoding + AllGather of fp8-encoded anchors.

Problem (hardcoded):
    N, M, D, C = 8192, 4096, 512, 100
    xe = mish(mish(x @ W1 + b1) @ W2 + b2)          [N, D]
    se = mish(mish(samples @ W1 + b1) @ W2 + b2)    [M, D]
    dist = sqrt(|xe|^2 + |se|^2 - 2 xe@se.T)          [N, M]
    out = log_softmax(tanh(dist @ Wp + bp), axis=1)   [N, C]

Sharding: core g handles x rows [1024g, 1024(g+1)) and encodes anchors
[512g, 512(g+1)); encoded anchors (scaled by -2, fp8e4) + |se|^2 (f16)
are AllGathered (fp8 payload halves the ring transfer vs bf16).

mish(v) = v * tanh(softplus(v)) is computed LUT-free via the exact
identity tanh(softplus(v)) = q/(q+2), q = w(w+2), w = e^v: one Exp on
ACT, one batched DVE pass for q, and a REGISTERED CUSTOM DVE OP
(MISH_TAIL_ANT) that evaluates vm*q/(q+2) in a single pass using the
BITWISE_NOT reciprocal seed + one Newton step (~0.2% rel err, far below
the fp8 distance quantization).  This removes the Ln/Tanh LUT passes
and ALL encoder activation-table switches.

The encoder is software-pipelined: the anchor chain (aL1, aL2, squares,
collective trigger) is emitted first on every engine queue, and x-phase
chunks are woven into its latency bubbles, so the collective arms at
~44us and the x side finishes under the mesh wait.

The distance GEMM runs on fp8e4 operands with DoubleRow perf mode
(2 contraction slabs per pass, 2x PE throughput).  Each anchor tile t
pairs BOTH 512-row x-chunks in one [128,1024] PSUM group: the |xe|^2
add (DVE) and the sqrt (ACT, with the f16 |se|^2 gathered alongside the
anchors riding in as the per-partition bias) run as single batched ops.
The perceptron GEMM (dist @ Wp, lag-2 behind the distance tiles) stays
float32r — full PE rate at free-dim 512, and dist ~ 32 is nearly
constant so Wp/dist rounding would bias whole output columns.

The AllGather is SPLIT INTO TWO HALF-ANCHOR MESHES (fp8 seA, -2x
scaled, plus an f16 |se|^2 row packed into two fp8 rows, 256 anchors
each): the distance loop runs all (rank, tile 0-1) pairs as soon as
mesh 1 lands, and mesh 2's transfer plus its reloads hide entirely
under that PE work (~26us), reaching the second-half tiles just in
time.  Reload descriptors are issued seA-then-s2 rank-ascending per
half, because the sync sequencer's blocking descriptor issue only
unblocks at each mesh's end and its issue order IS the
data-availability order.

Host-side packing: W1/W2/eT/Wp are pre-laid-out as [128, k-major]
arrays so SBUF tiles load with few large DMA descriptors (descriptor
issue is ~650ns each, serial on the sync sequencer); a "hot block"
(anchor columns + W1's first column block) heads the queue so the first
matmul fires ~6us after the sequencer boot barrier.
"""
import numpy as np
import ml_dtypes
from concourse import bass, bacc, tile, mybir, bass_utils, masks
from concourse import dve_ops as _dvo
from concourse.dve_spec import Spec as _Spec, Src0, Src1, C0, C1, C2, Bin as _Bin
from concourse.dve_spec import lower as _dve_lower
from concourse.dve_uop import AluOp as _DAlu, DveOpSpec as _DveOpSpec


def _register_mish_tail():
    """out = (in0*in1) * ~1/(in0+imm2): the mish tail vm*q/(q+2) in ONE
    DVE pass (BITWISE_NOT reciprocal seed + one Newton step, ~0.2% rel).
    Registered as a new custom-DVE op row; shas computed at import."""
    name = "MISH_TAIL_ANT"
    if name in _dvo._SUB_OPCODE_FOR_NAME:
        return next(o for o in _dvo.OPS if o.name == name)
    x = Src0 + C2
    nx = _Bin(_DAlu.BITWISE_NOT, x, x)
    y0 = nx * C0
    y1 = y0 * (C1 - x * y0)

    def _ref(in0, in1, c0, c1, c2):
        xx = in0.astype(np.float32) + np.float32(c2)
        nxx = (~xx.view(np.int32)).view(np.float32)
        yy0 = nxx * np.float32(c0)
        yy1 = yy0 * (np.float32(c1) - xx * yy0)
        return (in0.astype(np.float32) * in1.astype(np.float32)) * yy1

    spec = _Spec(body=(Src0 * Src1) * y1, reference=_ref)
    row = max(_dvo._SUB_OPCODE_FOR_NAME.values()) + 1
    shas = {}
    for ver in ("v3", "v4"):
        u = _dve_lower(spec, ver=ver)
        shas[ver] = _DveOpSpec(name=name, opcode=row, uops=u,
                               rd1_en=True).sha(ver)
    op = _dvo.DveOp(name, spec, subdim=False, uops_sha=shas,
                    perf_en={"v3": True, "v4": True})
    _dvo.OPS.append(op)
    _dvo.CUSTOM_DVE_SPECS[name] = spec
    _dvo._SUB_OPCODE_FOR_NAME[name] = row
    return op


MISH_TAIL = _register_mish_tail()
RECIP_C0, RECIP_C1 = -0.23549792, 2.0017324


def _patched_tables(arch):
    """Subset the ACT table sets (keeping dict order — act_func_set_id is
    positional) so Exp/Ln resolve only to natural_log_exp_and_others and
    Tanh only to exp_and_others. The default first-match choice alternates
    exp_and_others <-> natural_log on every exp/ln pair, paying a 1.3us
    table load each time."""
    from concourse.hw_specs import get_activation_tables as orig
    AFt = mybir.ActivationFunctionType
    out = {}
    for name, s in orig(arch).items():
        s = set(s)
        if name != "natural_log_exp_and_others":
            s.discard(AFt.Exp)
            s.discard(AFt.Ln)
            s.discard(AFt.Copy)
            s.discard(AFt.Identity)
        if name != "exp_and_others":
            s.discard(AFt.Tanh)
        out[name] = s
    return out

N, M, D, C = 8192, 4096, 512, 100
NCORES = 8
RPC = N // NCORES      # 1024 x-rows per core
MPC = M // NCORES      # 512 anchors encoded per core
KD = D // 128          # 4 contraction chunks of 128
NMT = M // 128         # 32 anchor tiles in the distance matmul
NRC = RPC // 512       # 2 row-chunks of 512
AGR = D + 2            # payload rows per rank: 512 seA + 2 (f16 s2)

F32 = mybir.dt.float32
F32R = mybir.dt.float32r
F16 = mybir.dt.float16
BF16 = mybir.dt.bfloat16
F8 = mybir.dt.float8e4
AF = mybir.ActivationFunctionType
ALU = mybir.AluOpType
DR = mybir.MatmulPerfMode.DoubleRow


def build_kernel():
    bacc.get_activation_tables = _patched_tables
    nc = bacc.Bacc("TRN2", target_bir_lowering=False, debug=False,
                   num_devices=NCORES)

    hot = nc.dram_tensor("hot", [128, KD * MPC + KD * 128], BF16,
                         kind="ExternalInput")
    eX = nc.dram_tensor("eX", [128, KD * RPC], BF16, kind="ExternalInput")
    W1p = nc.dram_tensor("W1p", [128, KD * D], BF16, kind="ExternalInput")
    W2p = nc.dram_tensor("W2p", [128, KD * D], BF16, kind="ExternalInput")
    bc = nc.dram_tensor("bc", [128, 3 * KD], F32, kind="ExternalInput")
    Wpp = nc.dram_tensor("Wpp", [128, NMT * C], F32, kind="ExternalInput")
    bp = nc.dram_tensor("bp", [1, C], F32, kind="ExternalInput")
    out = nc.dram_tensor("out", [RPC, C], F32, kind="ExternalOutput")

    with tile.TileContext(nc) as tc:
        _body(tc, hot, eX, W1p, W2p, bc, Wpp, bp, out)

    nc.compile()
    return nc


def _body(tc, hot, eX, W1p, W2p, bc, Wpp, bp, out):
    nc = tc.nc
    with (
        tc.tile_pool(name="const", bufs=1) as const,
        tc.tile_pool(name="wpool", bufs=1) as wpool,
        tc.tile_pool(name="spool", bufs=1) as spool,
        tc.tile_pool(name="xpool", bufs=1) as xpool,
        tc.tile_pool(name="gpool", bufs=1) as gpool,
        tc.tile_pool(name="mpool", bufs=2) as mpool,
        tc.tile_pool(name="dpool", bufs=8) as dpool,
        tc.tile_pool(name="zpool", bufs=2) as zpool,
        tc.tile_pool(name="ps", bufs=1, space="PSUM") as ps,
        tc.tile_pool(name="psz", bufs=1, space="PSUM") as psz,
        tc.tile_pool(name="dram", bufs=1, space="DRAM") as dram,
    ):
        # ---- need-ordered input loads; anchor-encode inputs head the
        # descriptor queue so the anchor->AllGather chain starts ASAP ----
        bc_sb = wpool.tile([128, 3 * KD], F32)
        nc.sync.dma_start(bc_sb[:], bc[:])
        b1c = bc_sb[:, 0:KD]
        b2c = bc_sb[:, KD:2 * KD]
        m2b2c = bc_sb[:, 2 * KD:3 * KD]
        eS_sb = spool.tile([128, KD, MPC], BF16)
        W1_sb = wpool.tile([128, KD, D], BF16)
        HB = KD * MPC + KD * 128
        nc.sync.dma_start(eS_sb[:, 0:2, :], hot[:, 0:2 * MPC])
        nc.sync.dma_start(eS_sb[:, 2:4, :], hot[:, 2 * MPC:4 * MPC])
        nc.sync.dma_start(
            W1_sb[:, :, 0:128],
            hot[:, KD * MPC:HB].rearrange("p (k f) -> p k f", k=KD))
        for k in range(KD):
            nc.sync.dma_start(W1_sb[:, k, 128:D],
                              W1p[:, D * k + 128:D * (k + 1)])
        W2_sb = wpool.tile([128, KD, D], BF16)
        eX_sb = xpool.tile([128, KD, RPC], BF16)
        for h in range(2):
            nc.sync.dma_start(W2_sb[:, 2 * h:2 * h + 2, :],
                              W2p[:, 2 * D * h:2 * D * (h + 1)])
        for h in range(2):
            nc.sync.dma_start(eX_sb[:, 2 * h:2 * h + 2, :],
                              eX[:, 2 * RPC * h:2 * RPC * (h + 1)])
        Wp_sb = wpool.tile([128, NMT, C], F32R)
        for h in range(2):
            nc.sync.dma_start(
                Wp_sb[:, h * (NMT // 2):(h + 1) * (NMT // 2), :],
                Wpp[:, h * NMT * C // 2:(h + 1) * NMT * C // 2].bitcast(F32R))
        bp_sb = wpool.tile([1, C], F32R)
        nc.sync.dma_start(bp_sb[:], bp[:].bitcast(F32R))

        # ---- constants (Copy resolves to exp_and_others, the same table
        # the encoder Exp uses, so startup costs one table load) ----
        ident = const.tile([C, C], F32)
        masks.make_identity(nc, ident[:])
        ones_f32 = const.tile([128, 1], F32)
        nc.vector.memset(ones_f32[:], 1.0)
        ones_col = const.tile([128, 1], BF16)    # lhsT for row-sum matmuls
        nc.scalar.activation(ones_col[:], ones_f32[:], AF.Copy)
        onesr_f32 = const.tile([1, 512], F32)
        nc.vector.memset(onesr_f32[:], 1.0)
        ones512 = const.tile([1, 512], F32R)     # rhs/lhsT for rank-1 terms
        nc.scalar.activation(ones512[:], onesr_f32[:], AF.Copy)

        def enc_head(u, vm, Wsb, bcol, abcol, src, w, vscale=1.0):
            """One 512-col chunk of a layer phase: matmuls + Exp (ACT) +
            v-stage (split: half ACT Identity with pre-scaled bias, half
            DVE tensor_scalar)."""
            ssl = slice(512 * w, 512 * (w + 1))
            for f in range(KD):
                vpt = ps.tile([128, 1024], F32, tag="mm", bufs=3)
                vps = vpt[:, 0:512]
                for k in range(KD):
                    nc.tensor.matmul(vps,
                                     Wsb[:, k, 128 * f:128 * (f + 1)],
                                     src[:, k, ssl],
                                     start=(k == 0), stop=(k == KD - 1))
                nc.scalar.activation(u[:, f, ssl], vps, AF.Exp,
                                     bias=bcol[:, f:f + 1])
                if f < 3:   # v-stage on ACT for 3/4 of the tiles
                    nc.scalar.activation(vm[:, f, ssl], vps, AF.Identity,
                                         bias=abcol[:, f:f + 1],
                                         scale=vscale)
                elif vscale == 1.0:
                    nc.vector.tensor_scalar_add(vm[:, f, ssl], vps,
                                                bcol[:, f:f + 1])
                else:
                    nc.vector.tensor_scalar(vm[:, f, ssl], vps,
                                            bcol[:, f:f + 1], vscale,
                                            op0=ALU.add, op1=ALU.mult)

        def enc_tail(dst, u, vm, width):
            """mish tail for a whole phase: batched q = (w+2)w (second
            512-chunk offloaded to the otherwise-idle GPSIMD) and the fused
            MISH_TAIL custom op on DVE."""
            q = mpool.tile([128, KD, width], BF16, tag="q")
            nc.vector.scalar_tensor_tensor(q[:, :, :], u[:, :, :], 2.0,
                                           u[:, :, :],
                                           op0=ALU.add, op1=ALU.mult)
            nc.vector._custom_dve(
                MISH_TAIL,
                out=dst[:, :, :].rearrange("p k m -> p (k m)"),
                in0=q[:, :, :].rearrange("p k m -> p (k m)"),
                in1=vm[:, :, :].rearrange("p k m -> p (k m)"),
                s0=RECIP_C0, s1=RECIP_C1, imm2=2.0)

        # ---- encoder, software-pipelined: the anchor chain (aL1, aL2,
        # squares, collective) is first in every engine queue; x-phase
        # chunks are woven into the anchor chain's latency bubbles (aL1's
        # DVE tail hides xL1-w0's ACT work, etc).  LUT-free mish means no
        # table switches, so the interleave costs nothing on ACT. ----
        h_se = spool.tile([128, KD, MPC], BF16)
        seA_bf = spool.tile([128, KD, MPC], BF16)
        h_xe = xpool.tile([128, KD, RPC], BF16)
        xe_bf = xpool.tile([128, KD, RPC], BF16)
        uA = mpool.tile([128, KD, MPC], BF16, tag="ua")
        vA = mpool.tile([128, KD, MPC], BF16, tag="va")
        uX = mpool.tile([128, KD, RPC], BF16, tag="ux")
        vX = mpool.tile([128, KD, RPC], BF16, tag="vx")

        enc_head(uA, vA, W1_sb, b1c, b1c, eS_sb, 0)           # aL1
        enc_tail(h_se, uA, vA, MPC)
        enc_head(uX, vX, W1_sb, b1c, b1c, eX_sb, 0)           # xL1 w0
        uA2 = mpool.tile([128, KD, MPC], BF16, tag="ua")
        vA2 = mpool.tile([128, KD, MPC], BF16, tag="va")
        enc_head(uA2, vA2, W2_sb, b2c, m2b2c, h_se, 0, vscale=-2.0)  # aL2
        enc_tail(seA_bf, uA2, vA2, MPC)

        # anchor tail: fp8 convert + s2 + the one collective
        seA8 = spool.tile([128, KD, MPC], F8)
        nc.scalar.activation(seA8[:, :, :], seA_bf[:, :, :], AF.Copy)
        sq_se = spool.tile([128, KD, MPC], BF16, tag="h_se")
        nc.vector.tensor_tensor(sq_se[:, :, :], seA_bf[:, :, :],
                                seA_bf[:, :, :], op=ALU.mult)
        s2pt = ps.tile([128, 1024], F32, tag="mm", bufs=3)
        for k in range(KD):
            nc.tensor.matmul(s2pt[0:1, 0:512], ones_col[:], sq_se[:, k, :],
                             start=(k == 0), stop=(k == KD - 1))
        s2row_sb = spool.tile([1, MPC], F16)
        nc.vector.tensor_scalar_mul(s2row_sb[:], s2pt[0:1, 0:512], 0.25)
        # two half-anchor AllGathers: the distance loop starts on the
        # (rank, tile 0-1) pairs as soon as mesh 1 lands; mesh 2 runs under
        # that PE work.  Payload per mesh: [512 seA rows + 2 f16-s2 rows,
        # 256 anchors] fp8.
        HM = MPC // 2
        ag_ins, ag_outs = [], []
        for h in range(2):
            agi = dram.tile([AGR, HM], F8, name=f"agi{h}")
            ago = dram.tile([NCORES * AGR, HM], F8, addr_space="Shared",
                            name=f"ago{h}")
            nc.sync.dma_start(
                agi[0:D, :].rearrange("(k p) m -> p k m", p=128),
                seA8[:, :, HM * h:HM * (h + 1)])
            nc.sync.dma_start(
                agi[D:D + 2, :].rearrange("(o a) b -> o (a b)", o=1),
                s2row_sb[:, HM * h:HM * (h + 1)].bitcast(F8))
            nc.gpsimd.collective_compute(
                "AllGather", ALU.bypass,
                replica_groups=[list(range(NCORES))],
                ins=[agi.opt()], outs=[ago.opt()])
            ag_ins.append(agi)
            ag_outs.append(ago)

        # ---- rest of the x columns (overlap the AllGather) ----
        enc_head(uX, vX, W1_sb, b1c, b1c, eX_sb, 1)           # xL1 w1
        enc_tail(h_xe, uX, vX, RPC)
        uX2 = mpool.tile([128, KD, RPC], BF16, tag="ux")
        vX2 = mpool.tile([128, KD, RPC], BF16, tag="vx")
        enc_head(uX2, vX2, W2_sb, b2c, b2c, h_xe, 0)          # xL2
        enc_head(uX2, vX2, W2_sb, b2c, b2c, h_xe, 1)
        enc_tail(xe_bf, uX2, vX2, RPC)
        xe8 = xpool.tile([128, KD, RPC], F8)
        for rc in range(NRC):
            csl = slice(512 * rc, 512 * (rc + 1))
            nc.scalar.activation(xe8[:, :, csl], xe_bf[:, :, csl], AF.Copy)

        # x2 broadcast tile: x2b[p, r] = |xe_r|^2 for every partition
        sq_xe = xpool.tile([128, KD, RPC], BF16, tag="h_xe")
        nc.vector.tensor_tensor(sq_xe[:, :, :], xe_bf[:, :, :],
                                xe_bf[:, :, :], op=ALU.mult)
        x2row_sb = xpool.tile([1, RPC], F32R)
        x2b_sb = xpool.tile([128, RPC], F32)
        xpt = ps.tile([128, 1024], F32, tag="mm", bufs=3)
        for rc in range(NRC):
            for k in range(KD):
                nc.tensor.matmul(xpt[0:1, 512 * rc:512 * (rc + 1)], ones_col[:],
                                 sq_xe[:, k, 512 * rc:512 * (rc + 1)],
                                 start=(k == 0), stop=(k == KD - 1),
                                 skip_group_check=True)
        nc.vector.tensor_copy(x2row_sb[:, :], xpt[0:1, :])
        xbt = ps.tile([128, 1024], F32, tag="mm", bufs=3)
        for rc in range(NRC):
            nc.tensor.matmul(xbt[:, 512 * rc:512 * (rc + 1)], ones512[:, :128],
                             x2row_sb[:, 512 * rc:512 * (rc + 1)],
                             start=True, stop=True, skip_group_check=True)
        nc.vector.tensor_copy(x2b_sb[:, :], xbt[:, :])

        # ---- load gathered anchors, rank-ascending (rank 0 gates the
        # first distance tile); two descriptors per rank, seA before s2.
        # The sync sequencer's blocking issue only unblocks at mesh-end,
        # so the issue order IS the availability order. ----
        seAg = [gpool.tile([128, KD, MPC], F8, name=f"seAg{g}")
                for g in range(NCORES)]
        s2all = gpool.tile([128, NCORES, KD], F16)
        for h in range(2):
            ago = ag_outs[h]
            for g in range(NCORES):
                nc.sync.dma_start(
                    seAg[g][:, :, HM * h:HM * (h + 1)],
                    ago[AGR * g:AGR * g + D, :].rearrange(
                        "(k p) m -> p k m", p=128))
                rows = ago[AGR * g + D:AGR * g + D + 2, :].bitcast(F16)
                nc.sync.dma_start(
                    s2all[:, g, 2 * h:2 * h + 2],
                    rows.rearrange("a (f p) -> p (a f)", f=1, p=128))

        # ---- main fused loop: each anchor tile t pairs BOTH row-chunks in
        # one [128,1024] psum group — the x2 add and the sqrt run as single
        # batched ops (x2b is exactly the contiguous free-dim operand, and
        # both halves share the per-partition |se|^2 sqrt bias). DoubleRow
        # fp8: 2 PE passes per 512-deep half. Perceptron lags one tile. ----
        zt_ps = [psz.tile([C, 512], F32, name=f"ztps{rc}") for rc in range(NRC)]
        zpre_sb = zpool.tile([128, 2 * NRC * 2, C], BF16, bufs=1)
        zt_sbs = []
        for rc in range(NRC):
            nc.tensor.matmul(zt_ps[rc][:], bp_sb[:], ones512[:],
                             start=True, stop=False, skip_group_check=True)
        dist_tiles = {}
        torder = [4 * g + tl for h in range(2)
                  for g in range(NCORES) for tl in (2 * h, 2 * h + 1)]
        for ti in range(NMT):
            t = torder[ti]
            g, tl = t // (MPC // 128), t % (MPC // 128)
            d2ps = ps.tile([128, 1024], F32, tag="mm", bufs=3)
            for rc in range(NRC):
                for kp in range(2):
                    nc.tensor.matmul(d2ps[:, 512 * rc:512 * (rc + 1)],
                                     seAg[g][:, 2 * kp:2 * kp + 2,
                                             128 * tl:128 * (tl + 1)],
                                     xe8[:, 2 * kp:2 * kp + 2,
                                         512 * rc:512 * (rc + 1)],
                                     start=(kp == 0), stop=(kp == 1),
                                     perf_mode=DR, skip_group_check=True)
            nc.vector.tensor_tensor(d2ps[:, :], d2ps[:, :], x2b_sb[:, :],
                                    op=ALU.add)
            distT = dpool.tile([128, 1024], F32R, bufs=4)
            nc.scalar.activation(distT[:, :], d2ps[:, :], AF.Sqrt,
                                 bias=s2all[:, g, tl:tl + 1])
            dist_tiles[t] = distT
            if ti >= 2:
                tp_ = torder[ti - 2]
                dprev = dist_tiles.pop(tp_)
                for rc in range(NRC):
                    nc.tensor.matmul(zt_ps[rc][:], Wp_sb[:, tp_, :],
                                     dprev[:, 512 * rc:512 * (rc + 1)],
                                     start=False, stop=False,
                                     skip_group_check=True)
        for ti in (NMT - 2, NMT - 1):
            tp_ = torder[ti]
            dprev = dist_tiles.pop(tp_)
            for rc in range(NRC):
                nc.tensor.matmul(zt_ps[rc][:], Wp_sb[:, tp_, :],
                                 dprev[:, 512 * rc:512 * (rc + 1)],
                                 start=False, stop=(ti == NMT - 1),
                                 skip_group_check=True)
        for rc in range(NRC):
            # epilogue part A: psum copy + transposes
            zt_sb = zpool.tile([C, 512], F32, bufs=2, tag="zt")
            nc.vector.tensor_copy(zt_sb[:], zt_ps[rc][:])
            zt_sbs.append(zt_sb)
            for j in range(4):
                zrt = ps.tile([128, 1024], F32, tag="mm", bufs=3)
                nc.tensor.matmul(zrt[0:128, 0:C],
                                 zt_sb[:, 128 * j:128 * (j + 1)],
                                 ident[:], is_transpose=True)
                nc.vector.tensor_copy(zpre_sb[:, 4 * rc + j, :],
                                      zrt[0:128, 0:C])

        # ---- epilogue: one batched tanh + log-softmax pass (tanh output
        # is in [-1,1] so no max-subtraction is needed) ----
        NT = 2 * NRC * 2  # 8 tiles of 128 rows
        zth_sb = zpool.tile([128, NT, C], BF16, bufs=1)
        nc.scalar.activation(zth_sb[:, :, :], zpre_sb[:, :, :], AF.Tanh)
        e_sb = zpool.tile([128, NT, C], BF16, bufs=1, tag="zpre_sb")
        nc.scalar.activation(e_sb[:, :, :], zth_sb[:, :, :], AF.Exp)
        ssum = zpool.tile([128, NT], F32, bufs=1)
        nc.vector.tensor_reduce(ssum[:], e_sb[:, :, :],
                                axis=mybir.AxisListType.X, op=ALU.add)
        lns = zpool.tile([128, NT], F32, bufs=1)
        nc.scalar.activation(lns[:], ssum[:], AF.Ln)
        o_sb = zpool.tile([128, NT, C], F32, bufs=1, tag="osb")
        for jj in range(NT):
            nc.vector.tensor_scalar(o_sb[:, jj, :], zth_sb[:, jj, :],
                                    lns[:, jj:jj + 1], None,
                                    op0=ALU.subtract)
        for h in range(2):
            nc.sync.dma_start(
                out[512 * h:512 * (h + 1), :].rearrange(
                    "(j p) c -> p j c", p=128),
                o_sb[:, 4 * h:4 * (h + 1), :])


_NC_CACHE = None


def _get_nc():
    global _NC_CACHE
    if _NC_CACHE is None:
        _NC_CACHE = build_kernel()
    return _NC_CACHE


def make_in_maps(x, samples, W1, b1, W2, b2, Wp, bp):
    bf = ml_dtypes.bfloat16
    x = np.asarray(x, dtype=np.float32)
    samples = np.asarray(samples, dtype=np.float32)

    def kpack(w):  # [D, cols] -> [128, KD, cols] with [p, k, c] = w[128k+p, c]
        w = np.asarray(w, dtype=np.float32)
        return np.ascontiguousarray(
            w.reshape(KD, 128, -1).transpose(1, 0, 2).reshape(128, -1))

    W1b = kpack(W1).astype(bf)
    W2b = kpack(W2).astype(bf)
    Wpc = np.ascontiguousarray(
        np.asarray(Wp, dtype=np.float32).reshape(NMT, 128, C)
        .transpose(1, 0, 2).reshape(128, NMT * C))
    b1c = np.asarray(b1, dtype=np.float32).reshape(KD, 128).T
    b2c = np.asarray(b2, dtype=np.float32).reshape(KD, 128).T
    bcc = np.ascontiguousarray(np.concatenate([b1c, b2c, -2.0 * b2c], axis=1))
    bpc = np.ascontiguousarray(np.asarray(bp, dtype=np.float32).reshape(1, C))
    W1f0 = np.ascontiguousarray(
        W1b.reshape(128, KD, D)[:, :, 0:128].reshape(128, KD * 128))
    in_maps = []
    for g in range(NCORES):
        sT_g = kpack(samples[MPC * g:MPC * (g + 1), :].T).astype(bf)
        xT_g = kpack(x[RPC * g:RPC * (g + 1), :].T).astype(bf)
        hot = np.ascontiguousarray(np.concatenate([sT_g, W1f0], axis=1))
        in_maps.append({
            "hot": hot,
            "eX": np.ascontiguousarray(xT_g),
            "W1p": W1b, "W2p": W2b, "bc": bcc,
            "Wpp": Wpc, "bp": bpc,
        })
    return in_maps


def run(in_maps, trace=False):
    nc = _get_nc()
    res = bass_utils.run_bass_kernel_spmd(nc, in_maps,
                                          core_ids=list(range(NCORES)),
                                          trace=trace)
    outp = np.concatenate([res.results[g]["out"] for g in range(NCORES)],
                          axis=0).astype(np.float32)
    return outp, res


def kernel(x, samples, W1, b1, W2, b2, Wp, bp):
    in_maps = make_in_maps(x, samples, W1, b1, W2, b2, Wp, bp)
    outp, _ = run(in_maps, trace=False)
    return outp


# revision 27
# speedup vs baseline: 1.6123x; 1.6123x over previous
"""AnchorDML Trainium2 kernel: 8-core SPMD, data-parallel over x rows with
sharded anchor encoding + AllGather of fp8-encoded anchors.

Problem (hardcoded):
    N, M, D, C = 8192, 4096, 512, 100
    xe = mish(mish(x @ W1 + b1) @ W2 + b2)          [N, D]
    se = mish(mish(samples @ W1 + b1) @ W2 + b2)    [M, D]
    dist = sqrt(|xe|^2 + |se|^2 - 2 xe@se.T)          [N, M]
    out = log_softmax(tanh(dist @ Wp + bp), axis=1)   [N, C]

Sharding: core g handles x rows [1024g, 1024(g+1)) and encodes anchors
[512g, 512(g+1)); encoded anchors (scaled by -2, fp8e4) + |se|^2 (f16)
are AllGathered (fp8 payload halves the ring transfer vs bf16).

mish(v) = v * tanh(softplus(v)) is computed LUT-free via the exact
identity tanh(softplus(v)) = q/(q+2), q = w(w+2), w = e^v: one Exp on
ACT, one batched DVE pass for q, and a REGISTERED CUSTOM DVE OP
(MISH_TAIL_ANT) that evaluates vm*q/(q+2) in a single pass using the
BITWISE_NOT reciprocal seed + one Newton step (~0.2% rel err, far below
the fp8 distance quantization).  This removes the Ln/Tanh LUT passes
and ALL encoder activation-table switches.

The encoder is software-pipelined: the anchor chain (aL1, aL2, squares,
collective trigger) is emitted first on every engine queue, and x-phase
chunks are woven into its latency bubbles, so the collective arms at
~44us and the x side finishes under the mesh wait.

The distance GEMM runs on fp8e4 operands with DoubleRow perf mode
(2 contraction slabs per pass, 2x PE throughput).  Each anchor tile t
pairs BOTH 512-row x-chunks in one [128,1024] PSUM group: the |xe|^2
add (DVE) and the sqrt (ACT, with the f16 |se|^2 gathered alongside the
anchors riding in as the per-partition bias) run as single batched ops.
The perceptron GEMM (dist @ Wp, lag-2 behind the distance tiles) stays
float32r — full PE rate at free-dim 512, and dist ~ 32 is nearly
constant so Wp/dist rounding would bias whole output columns.

The AllGather is SPLIT INTO TWO HALF-ANCHOR MESHES (fp8 seA, -2x
scaled, plus an f16 |se|^2 row packed into two fp8 rows, 256 anchors
each): the distance loop runs all (rank, tile 0-1) pairs as soon as
mesh 1 lands, and mesh 2's transfer plus its reloads hide entirely
under that PE work (~26us), reaching the second-half tiles just in
time.  Reload descriptors are issued seA-then-s2 rank-ascending per
half, because the sync sequencer's blocking descriptor issue only
unblocks at each mesh's end and its issue order IS the
data-availability order.

Host-side packing: W1/W2/eT/Wp are pre-laid-out as [128, k-major]
arrays so SBUF tiles load with few large DMA descriptors (descriptor
issue is ~650ns each, serial on the sync sequencer); a "hot block"
(anchor columns + W1's first column block) heads the queue so the first
matmul fires ~6us after the sequencer boot barrier.
"""
import numpy as np
import ml_dtypes
from concourse import bass, bacc, tile, mybir, bass_utils, masks
from concourse import dve_ops as _dvo
from concourse.dve_spec import Spec as _Spec, Src0, Src1, C0, C1, C2, Bin as _Bin
from concourse.dve_spec import lower as _dve_lower
from concourse.dve_uop import AluOp as _DAlu, DveOpSpec as _DveOpSpec


def _register_mish_tail():
    """out = (in0*in1) * ~1/(in0+imm2): the mish tail vm*q/(q+2) in ONE
    DVE pass (BITWISE_NOT reciprocal seed + one Newton step, ~0.2% rel).
    Registered as a new custom-DVE op row; shas computed at import."""
    name = "MISH_TAIL_ANT"
    if name in _dvo._SUB_OPCODE_FOR_NAME:
        return next(o for o in _dvo.OPS if o.name == name)
    x = Src0 + C2
    nx = _Bin(_DAlu.BITWISE_NOT, x, x)
    y0 = nx * C0
    y1 = y0 * (C1 - x * y0)

    def _ref(in0, in1, c0, c1, c2):
        xx = in0.astype(np.float32) + np.float32(c2)
        nxx = (~xx.view(np.int32)).view(np.float32)
        yy0 = nxx * np.float32(c0)
        yy1 = yy0 * (np.float32(c1) - xx * yy0)
        return (in0.astype(np.float32) * in1.astype(np.float32)) * yy1

    spec = _Spec(body=(Src0 * Src1) * y1, reference=_ref)
    row = max(_dvo._SUB_OPCODE_FOR_NAME.values()) + 1
    shas = {}
    for ver in ("v3", "v4"):
        u = _dve_lower(spec, ver=ver)
        shas[ver] = _DveOpSpec(name=name, opcode=row, uops=u,
                               rd1_en=True).sha(ver)
    op = _dvo.DveOp(name, spec, subdim=False, uops_sha=shas,
                    perf_en={"v3": True, "v4": True})
    _dvo.OPS.append(op)
    _dvo.CUSTOM_DVE_SPECS[name] = spec
    _dvo._SUB_OPCODE_FOR_NAME[name] = row
    return op


MISH_TAIL = _register_mish_tail()
RECIP_C0, RECIP_C1 = -0.23549792, 2.0017324


def _patched_tables(arch):
    """Subset the ACT table sets (keeping dict order — act_func_set_id is
    positional) so Exp/Ln resolve only to natural_log_exp_and_others and
    Tanh only to exp_and_others. The default first-match choice alternates
    exp_and_others <-> natural_log on every exp/ln pair, paying a 1.3us
    table load each time."""
    from concourse.hw_specs import get_activation_tables as orig
    AFt = mybir.ActivationFunctionType
    out = {}
    for name, s in orig(arch).items():
        s = set(s)
        if name != "natural_log_exp_and_others":
            s.discard(AFt.Exp)
            s.discard(AFt.Ln)
            s.discard(AFt.Copy)
            s.discard(AFt.Identity)
        if name != "exp_and_others":
            s.discard(AFt.Tanh)
        out[name] = s
    return out

N, M, D, C = 8192, 4096, 512, 100
NCORES = 8
RPC = N // NCORES      # 1024 x-rows per core
MPC = M // NCORES      # 512 anchors encoded per core
KD = D // 128          # 4 contraction chunks of 128
NMT = M // 128         # 32 anchor tiles in the distance matmul
NRC = RPC // 512       # 2 row-chunks of 512
AGR = D + 2            # payload rows per rank: 512 seA + 2 (f16 s2)

F32 = mybir.dt.float32
F32R = mybir.dt.float32r
F16 = mybir.dt.float16
BF16 = mybir.dt.bfloat16
F8 = mybir.dt.float8e4
AF = mybir.ActivationFunctionType
ALU = mybir.AluOpType
DR = mybir.MatmulPerfMode.DoubleRow


def build_kernel():
    bacc.get_activation_tables = _patched_tables
    nc = bacc.Bacc("TRN2", target_bir_lowering=False, debug=False,
                   num_devices=NCORES)

    hot = nc.dram_tensor("hot", [128, KD * MPC + KD * 128], BF16,
                         kind="ExternalInput")
    eX = nc.dram_tensor("eX", [128, KD * RPC], BF16, kind="ExternalInput")
    W1p = nc.dram_tensor("W1p", [128, KD * D], BF16, kind="ExternalInput")
    W2p = nc.dram_tensor("W2p", [128, KD * D], BF16, kind="ExternalInput")
    bc = nc.dram_tensor("bc", [128, 3 * KD], F32, kind="ExternalInput")
    Wpp = nc.dram_tensor("Wpp", [128, NMT * C], F32, kind="ExternalInput")
    bp = nc.dram_tensor("bp", [1, C], F32, kind="ExternalInput")
    out = nc.dram_tensor("out", [RPC, C], F32, kind="ExternalOutput")

    with tile.TileContext(nc) as tc:
        _body(tc, hot, eX, W1p, W2p, bc, Wpp, bp, out)

    nc.compile()
    return nc


def _body(tc, hot, eX, W1p, W2p, bc, Wpp, bp, out):
    nc = tc.nc
    with (
        tc.tile_pool(name="const", bufs=1) as const,
        tc.tile_pool(name="wpool", bufs=1) as wpool,
        tc.tile_pool(name="spool", bufs=1) as spool,
        tc.tile_pool(name="xpool", bufs=1) as xpool,
        tc.tile_pool(name="gpool", bufs=1) as gpool,
        tc.tile_pool(name="mpool", bufs=2) as mpool,
        tc.tile_pool(name="dpool", bufs=8) as dpool,
        tc.tile_pool(name="zpool", bufs=2) as zpool,
        tc.tile_pool(name="ps", bufs=1, space="PSUM") as ps,
        tc.tile_pool(name="psz", bufs=1, space="PSUM") as psz,
        tc.tile_pool(name="dram", bufs=1, space="DRAM") as dram,
    ):
        # ---- need-ordered input loads; anchor-encode inputs head the
        # descriptor queue so the anchor->AllGather chain starts ASAP ----
        bc_sb = wpool.tile([128, 3 * KD], F32)
        nc.sync.dma_start(bc_sb[:], bc[:])
        b1c = bc_sb[:, 0:KD]
        b2c = bc_sb[:, KD:2 * KD]
        m2b2c = bc_sb[:, 2 * KD:3 * KD]
        eS_sb = spool.tile([128, KD, MPC], BF16)
        W1_sb = wpool.tile([128, KD, D], BF16)
        HB = KD * MPC + KD * 128
        nc.sync.dma_start(eS_sb[:, 0:2, :], hot[:, 0:2 * MPC])
        nc.sync.dma_start(eS_sb[:, 2:4, :], hot[:, 2 * MPC:4 * MPC])
        nc.sync.dma_start(
            W1_sb[:, :, 0:128],
            hot[:, KD * MPC:HB].rearrange("p (k f) -> p k f", k=KD))
        for k in range(KD):
            nc.sync.dma_start(W1_sb[:, k, 128:D],
                              W1p[:, D * k + 128:D * (k + 1)])
        W2_sb = wpool.tile([128, KD, D], BF16)
        eX_sb = xpool.tile([128, KD, RPC], BF16)
        for h in range(2):
            nc.sync.dma_start(W2_sb[:, 2 * h:2 * h + 2, :],
                              W2p[:, 2 * D * h:2 * D * (h + 1)])
        for h in range(2):
            nc.sync.dma_start(eX_sb[:, 2 * h:2 * h + 2, :],
                              eX[:, 2 * RPC * h:2 * RPC * (h + 1)])
        Wp_sb = wpool.tile([128, NMT, C], F32R)
        for h in range(2):
            nc.sync.dma_start(
                Wp_sb[:, h * (NMT // 2):(h + 1) * (NMT // 2), :],
                Wpp[:, h * NMT * C // 2:(h + 1) * NMT * C // 2].bitcast(F32R))
        bp_sb = wpool.tile([1, C], F32R)
        nc.sync.dma_start(bp_sb[:], bp[:].bitcast(F32R))

        # ---- constants (Copy resolves to exp_and_others, the same table
        # the encoder Exp uses, so startup costs one table load) ----
        ident = const.tile([C, C], F32)
        masks.make_identity(nc, ident[:])
        ones_f32 = const.tile([128, 1], F32)
        nc.vector.memset(ones_f32[:], 1.0)
        ones_col = const.tile([128, 1], BF16)    # lhsT for row-sum matmuls
        nc.scalar.activation(ones_col[:], ones_f32[:], AF.Copy)
        onesr_f32 = const.tile([1, 512], F32)
        nc.vector.memset(onesr_f32[:], 1.0)
        ones512 = const.tile([1, 512], F32R)     # rhs/lhsT for rank-1 terms
        nc.scalar.activation(ones512[:], onesr_f32[:], AF.Copy)

        def enc_head(u, vm, Wsb, bcol, abcol, src, w, vscale=1.0,
                     act_vst=3):
            """One 512-col chunk of a layer phase: matmuls + Exp (ACT) +
            v-stage.  act_vst = how many of the KD tiles put the v-stage on
            ACT (Identity with pre-scaled bias) vs DVE: x-phases use 3 for
            throughput balance; the latency-bound anchor phases use 0 so
            the DVE v-stages fully overlap the serial ACT exp chain."""
            ssl = slice(512 * w, 512 * (w + 1))
            for f in range(KD):
                vpt = ps.tile([128, 1024], F32, tag="mm", bufs=3)
                vps = vpt[:, 0:512]
                for k in range(KD):
                    nc.tensor.matmul(vps,
                                     Wsb[:, k, 128 * f:128 * (f + 1)],
                                     src[:, k, ssl],
                                     start=(k == 0), stop=(k == KD - 1))
                nc.scalar.activation(u[:, f, ssl], vps, AF.Exp,
                                     bias=bcol[:, f:f + 1])
                if f < act_vst:   # v-stage on ACT
                    nc.scalar.activation(vm[:, f, ssl], vps, AF.Identity,
                                         bias=abcol[:, f:f + 1],
                                         scale=vscale)
                elif vscale == 1.0:
                    nc.vector.tensor_scalar_add(vm[:, f, ssl], vps,
                                                bcol[:, f:f + 1])
                else:
                    nc.vector.tensor_scalar(vm[:, f, ssl], vps,
                                            bcol[:, f:f + 1], vscale,
                                            op0=ALU.add, op1=ALU.mult)

        def enc_tail(dst, u, vm, width):
            """mish tail for a whole phase: batched q = (w+2)w (second
            512-chunk offloaded to the otherwise-idle GPSIMD) and the fused
            MISH_TAIL custom op on DVE."""
            q = mpool.tile([128, KD, width], BF16, tag="q")
            nc.vector.scalar_tensor_tensor(q[:, :, :], u[:, :, :], 2.0,
                                           u[:, :, :],
                                           op0=ALU.add, op1=ALU.mult)
            nc.vector._custom_dve(
                MISH_TAIL,
                out=dst[:, :, :].rearrange("p k m -> p (k m)"),
                in0=q[:, :, :].rearrange("p k m -> p (k m)"),
                in1=vm[:, :, :].rearrange("p k m -> p (k m)"),
                s0=RECIP_C0, s1=RECIP_C1, imm2=2.0)

        # ---- encoder, software-pipelined: the anchor chain (aL1, aL2,
        # squares, collective) is first in every engine queue; x-phase
        # chunks are woven into the anchor chain's latency bubbles (aL1's
        # DVE tail hides xL1-w0's ACT work, etc).  LUT-free mish means no
        # table switches, so the interleave costs nothing on ACT. ----
        h_se = spool.tile([128, KD, MPC], BF16)
        seA_bf = spool.tile([128, KD, MPC], BF16)
        h_xe = xpool.tile([128, KD, RPC], BF16)
        xe_bf = xpool.tile([128, KD, RPC], BF16)
        uA = mpool.tile([128, KD, MPC], BF16, tag="ua")
        vA = mpool.tile([128, KD, MPC], BF16, tag="va")
        uX = mpool.tile([128, KD, RPC], BF16, tag="ux")
        vX = mpool.tile([128, KD, RPC], BF16, tag="vx")

        enc_head(uA, vA, W1_sb, b1c, b1c, eS_sb, 0, act_vst=0)  # aL1
        enc_tail(h_se, uA, vA, MPC)
        enc_head(uX, vX, W1_sb, b1c, b1c, eX_sb, 0)           # xL1 w0
        uA2 = mpool.tile([128, KD, MPC], BF16, tag="ua")
        vA2 = mpool.tile([128, KD, MPC], BF16, tag="va")
        enc_head(uA2, vA2, W2_sb, b2c, m2b2c, h_se, 0, vscale=-2.0,
                 act_vst=0)  # aL2
        enc_tail(seA_bf, uA2, vA2, MPC)

        # anchor tail: fp8 convert + s2 + the one collective
        seA8 = spool.tile([128, KD, MPC], F8)
        nc.scalar.activation(seA8[:, :, :], seA_bf[:, :, :], AF.Copy)
        sq_se = spool.tile([128, KD, MPC], BF16, tag="h_se")
        nc.vector.tensor_tensor(sq_se[:, :, :], seA_bf[:, :, :],
                                seA_bf[:, :, :], op=ALU.mult)
        s2pt = ps.tile([128, 1024], F32, tag="mm", bufs=3)
        for k in range(KD):
            nc.tensor.matmul(s2pt[0:1, 0:512], ones_col[:], sq_se[:, k, :],
                             start=(k == 0), stop=(k == KD - 1))
        s2row_sb = spool.tile([1, MPC], F16)
        nc.vector.tensor_scalar_mul(s2row_sb[:], s2pt[0:1, 0:512], 0.25)
        # two half-anchor AllGathers: the distance loop starts on the
        # (rank, tile 0-1) pairs as soon as mesh 1 lands; mesh 2 runs under
        # that PE work.  Payload per mesh: [512 seA rows + 2 f16-s2 rows,
        # 256 anchors] fp8.
        HM = MPC // 2
        ag_ins, ag_outs = [], []
        for h in range(2):
            agi = dram.tile([AGR, HM], F8, name=f"agi{h}")
            ago = dram.tile([NCORES * AGR, HM], F8, addr_space="Shared",
                            name=f"ago{h}")
            nc.sync.dma_start(
                agi[0:D, :].rearrange("(k p) m -> p k m", p=128),
                seA8[:, :, HM * h:HM * (h + 1)])
            nc.sync.dma_start(
                agi[D:D + 2, :].rearrange("(o a) b -> o (a b)", o=1),
                s2row_sb[:, HM * h:HM * (h + 1)].bitcast(F8))
            nc.gpsimd.collective_compute(
                "AllGather", ALU.bypass,
                replica_groups=[list(range(NCORES))],
                ins=[agi.opt()], outs=[ago.opt()])
            ag_ins.append(agi)
            ag_outs.append(ago)

        # ---- rest of the x columns (overlap the AllGather) ----
        enc_head(uX, vX, W1_sb, b1c, b1c, eX_sb, 1)           # xL1 w1
        enc_tail(h_xe, uX, vX, RPC)
        uX2 = mpool.tile([128, KD, RPC], BF16, tag="ux")
        vX2 = mpool.tile([128, KD, RPC], BF16, tag="vx")
        enc_head(uX2, vX2, W2_sb, b2c, b2c, h_xe, 0)          # xL2
        enc_head(uX2, vX2, W2_sb, b2c, b2c, h_xe, 1)
        enc_tail(xe_bf, uX2, vX2, RPC)
        xe8 = xpool.tile([128, KD, RPC], F8)
        for rc in range(NRC):
            csl = slice(512 * rc, 512 * (rc + 1))
            nc.scalar.activation(xe8[:, :, csl], xe_bf[:, :, csl], AF.Copy)

        # x2 broadcast tile: x2b[p, r] = |xe_r|^2 for every partition
        sq_xe = xpool.tile([128, KD, RPC], BF16, tag="h_xe")
        nc.vector.tensor_tensor(sq_xe[:, :, :], xe_bf[:, :, :],
                                xe_bf[:, :, :], op=ALU.mult)
        x2row_sb = xpool.tile([1, RPC], F32R)
        x2b_sb = xpool.tile([128, RPC], F32)
        xpt = ps.tile([128, 1024], F32, tag="mm", bufs=3)
        for rc in range(NRC):
            for k in range(KD):
                nc.tensor.matmul(xpt[0:1, 512 * rc:512 * (rc + 1)], ones_col[:],
                                 sq_xe[:, k, 512 * rc:512 * (rc + 1)],
                                 start=(k == 0), stop=(k == KD - 1),
                                 skip_group_check=True)
        nc.vector.tensor_copy(x2row_sb[:, :], xpt[0:1, :])
        xbt = ps.tile([128, 1024], F32, tag="mm", bufs=3)
        for rc in range(NRC):
            nc.tensor.matmul(xbt[:, 512 * rc:512 * (rc + 1)], ones512[:, :128],
                             x2row_sb[:, 512 * rc:512 * (rc + 1)],
                             start=True, stop=True, skip_group_check=True)
        nc.vector.tensor_copy(x2b_sb[:, :], xbt[:, :])

        # ---- load gathered anchors, rank-ascending (rank 0 gates the
        # first distance tile); two descriptors per rank, seA before s2.
        # The sync sequencer's blocking issue only unblocks at mesh-end,
        # so the issue order IS the availability order. ----
        seAg = [gpool.tile([128, KD, MPC], F8, name=f"seAg{g}")
                for g in range(NCORES)]
        s2all = gpool.tile([128, NCORES, KD], F16)
        for h in range(2):
            ago = ag_outs[h]
            for g in range(NCORES):
                nc.sync.dma_start(
                    seAg[g][:, :, HM * h:HM * (h + 1)],
                    ago[AGR * g:AGR * g + D, :].rearrange(
                        "(k p) m -> p k m", p=128))
                rows = ago[AGR * g + D:AGR * g + D + 2, :].bitcast(F16)
                nc.sync.dma_start(
                    s2all[:, g, 2 * h:2 * h + 2],
                    rows.rearrange("a (f p) -> p (a f)", f=1, p=128))

        # ---- main fused loop: each anchor tile t pairs BOTH row-chunks in
        # one [128,1024] psum group — the x2 add and the sqrt run as single
        # batched ops (x2b is exactly the contiguous free-dim operand, and
        # both halves share the per-partition |se|^2 sqrt bias). DoubleRow
        # fp8: 2 PE passes per 512-deep half. Perceptron lags one tile. ----
        zt_ps = [psz.tile([C, 512], F32, name=f"ztps{rc}") for rc in range(NRC)]
        zpre_sb = zpool.tile([128, 2 * NRC * 2, C], BF16, bufs=1)
        zt_sbs = []
        for rc in range(NRC):
            nc.tensor.matmul(zt_ps[rc][:], bp_sb[:], ones512[:],
                             start=True, stop=False, skip_group_check=True)
        dist_tiles = {}
        torder = [4 * g + tl for h in range(2)
                  for g in range(NCORES) for tl in (2 * h, 2 * h + 1)]
        for ti in range(NMT):
            t = torder[ti]
            g, tl = t // (MPC // 128), t % (MPC // 128)
            d2ps = ps.tile([128, 1024], F32, tag="mm", bufs=3)
            for rc in range(NRC):
                for kp in range(2):
                    nc.tensor.matmul(d2ps[:, 512 * rc:512 * (rc + 1)],
                                     seAg[g][:, 2 * kp:2 * kp + 2,
                                             128 * tl:128 * (tl + 1)],
                                     xe8[:, 2 * kp:2 * kp + 2,
                                         512 * rc:512 * (rc + 1)],
                                     start=(kp == 0), stop=(kp == 1),
                                     perf_mode=DR, skip_group_check=True)
            nc.vector.tensor_tensor(d2ps[:, :], d2ps[:, :], x2b_sb[:, :],
                                    op=ALU.add)
            distT = dpool.tile([128, 1024], F32R, bufs=4)
            nc.scalar.activation(distT[:, :], d2ps[:, :], AF.Sqrt,
                                 bias=s2all[:, g, tl:tl + 1])
            dist_tiles[t] = distT
            if ti >= 2:
                tp_ = torder[ti - 2]
                dprev = dist_tiles.pop(tp_)
                for rc in range(NRC):
                    nc.tensor.matmul(zt_ps[rc][:], Wp_sb[:, tp_, :],
                                     dprev[:, 512 * rc:512 * (rc + 1)],
                                     start=False, stop=False,
                                     skip_group_check=True)
        for ti in (NMT - 2, NMT - 1):
            tp_ = torder[ti]
            dprev = dist_tiles.pop(tp_)
            for rc in range(NRC):
                nc.tensor.matmul(zt_ps[rc][:], Wp_sb[:, tp_, :],
                                 dprev[:, 512 * rc:512 * (rc + 1)],
                                 start=False, stop=(ti == NMT - 1),
                                 skip_group_check=True)
        for rc in range(NRC):
            # epilogue part A: psum copy + transposes
            zt_sb = zpool.tile([C, 512], F32, bufs=2, tag="zt")
            nc.vector.tensor_copy(zt_sb[:], zt_ps[rc][:])
            zt_sbs.append(zt_sb)
            for j in range(4):
                zrt = ps.tile([128, 1024], F32, tag="mm", bufs=3)
                nc.tensor.matmul(zrt[0:128, 0:C],
                                 zt_sb[:, 128 * j:128 * (j + 1)],
                                 ident[:], is_transpose=True)
                nc.vector.tensor_copy(zpre_sb[:, 4 * rc + j, :],
                                      zrt[0:128, 0:C])

        # ---- epilogue: one batched tanh + log-softmax pass (tanh output
        # is in [-1,1] so no max-subtraction is needed) ----
        NT = 2 * NRC * 2  # 8 tiles of 128 rows
        zth_sb = zpool.tile([128, NT, C], BF16, bufs=1)
        nc.scalar.activation(zth_sb[:, :, :], zpre_sb[:, :, :], AF.Tanh)
        e_sb = zpool.tile([128, NT, C], BF16, bufs=1, tag="zpre_sb")
        nc.scalar.activation(e_sb[:, :, :], zth_sb[:, :, :], AF.Exp)
        ssum = zpool.tile([128, NT], F32, bufs=1)
        nc.vector.tensor_reduce(ssum[:], e_sb[:, :, :],
                                axis=mybir.AxisListType.X, op=ALU.add)
        lns = zpool.tile([128, NT], F32, bufs=1)
        nc.scalar.activation(lns[:], ssum[:], AF.Ln)
        o_sb = zpool.tile([128, NT, C], F32, bufs=1, tag="osb")
        for jj in range(NT):
            nc.vector.tensor_scalar(o_sb[:, jj, :], zth_sb[:, jj, :],
                                    lns[:, jj:jj + 1], None,
                                    op0=ALU.subtract)
        for h in range(2):
            nc.sync.dma_start(
                out[512 * h:512 * (h + 1), :].rearrange(
                    "(j p) c -> p j c", p=128),
                o_sb[:, 4 * h:4 * (h + 1), :])


_NC_CACHE = None


def _get_nc():
    global _NC_CACHE
    if _NC_CACHE is None:
        _NC_CACHE = build_kernel()
    return _NC_CACHE


def make_in_maps(x, samples, W1, b1, W2, b2, Wp, bp):
    bf = ml_dtypes.bfloat16
    x = np.asarray(x, dtype=np.float32)
    samples = np.asarray(samples, dtype=np.float32)

    def kpack(w):  # [D, cols] -> [128, KD, cols] with [p, k, c] = w[128k+p, c]
        w = np.asarray(w, dtype=np.float32)
        return np.ascontiguousarray(
            w.reshape(KD, 128, -1).transpose(1, 0, 2).reshape(128, -1))

    W1b = kpack(W1).astype(bf)
    W2b = kpack(W2).astype(bf)
    Wpc = np.ascontiguousarray(
        np.asarray(Wp, dtype=np.float32).reshape(NMT, 128, C)
        .transpose(1, 0, 2).reshape(128, NMT * C))
    b1c = np.asarray(b1, dtype=np.float32).reshape(KD, 128).T
    b2c = np.asarray(b2, dtype=np.float32).reshape(KD, 128).T
    bcc = np.ascontiguousarray(np.concatenate([b1c, b2c, -2.0 * b2c], axis=1))
    bpc = np.ascontiguousarray(np.asarray(bp, dtype=np.float32).reshape(1, C))
    W1f0 = np.ascontiguousarray(
        W1b.reshape(128, KD, D)[:, :, 0:128].reshape(128, KD * 128))
    in_maps = []
    for g in range(NCORES):
        sT_g = kpack(samples[MPC * g:MPC * (g + 1), :].T).astype(bf)
        xT_g = kpack(x[RPC * g:RPC * (g + 1), :].T).astype(bf)
        hot = np.ascontiguousarray(np.concatenate([sT_g, W1f0], axis=1))
        in_maps.append({
            "hot": hot,
            "eX": np.ascontiguousarray(xT_g),
            "W1p": W1b, "W2p": W2b, "bc": bcc,
            "Wpp": Wpc, "bp": bpc,
        })
    return in_maps


def run(in_maps, trace=False):
    nc = _get_nc()
    res = bass_utils.run_bass_kernel_spmd(nc, in_maps,
                                          core_ids=list(range(NCORES)),
                                          trace=trace)
    outp = np.concatenate([res.results[g]["out"] for g in range(NCORES)],
                          axis=0).astype(np.float32)
    return outp, res


def kernel(x, samples, W1, b1, W2, b2, Wp, bp):
    in_maps = make_in_maps(x, samples, W1, b1, W2, b2, Wp, bp)
    outp, _ = run(in_maps, trace=False)
    return outp


# revision 28
# speedup vs baseline: 1.6290x; 1.0103x over previous
"""AnchorDML Trainium2 kernel: 8-core SPMD, data-parallel over x rows with
sharded anchor encoding + AllGather of fp8-encoded anchors.

Problem (hardcoded):
    N, M, D, C = 8192, 4096, 512, 100
    xe = mish(mish(x @ W1 + b1) @ W2 + b2)          [N, D]
    se = mish(mish(samples @ W1 + b1) @ W2 + b2)    [M, D]
    dist = sqrt(|xe|^2 + |se|^2 - 2 xe@se.T)          [N, M]
    out = log_softmax(tanh(dist @ Wp + bp), axis=1)   [N, C]

Sharding: core g handles x rows [1024g, 1024(g+1)) and encodes anchors
[512g, 512(g+1)); encoded anchors (scaled by -2, fp8e4) + |se|^2 (f16)
are AllGathered (fp8 payload halves the ring transfer vs bf16).

mish(v) = v * tanh(softplus(v)) is computed LUT-free via the exact
identity tanh(softplus(v)) = q/(q+2), q = w(w+2), w = e^v: one Exp on
ACT, one batched DVE pass for q, and a REGISTERED CUSTOM DVE OP
(MISH_TAIL_ANT) that evaluates vm*q/(q+2) in a single pass using the
BITWISE_NOT reciprocal seed + one Newton step (~0.2% rel err, far below
the fp8 distance quantization).  This removes the Ln/Tanh LUT passes
and ALL encoder activation-table switches.

The encoder is software-pipelined: the anchor chain (aL1, aL2, squares,
collective trigger) is emitted first on every engine queue, and x-phase
chunks are woven into its latency bubbles, so the collective arms at
~44us and the x side finishes under the mesh wait.

The distance GEMM runs on fp8e4 operands with DoubleRow perf mode
(2 contraction slabs per pass, 2x PE throughput).  Each anchor tile t
pairs BOTH 512-row x-chunks in one [128,1024] PSUM group: the |xe|^2
add (DVE) and the sqrt (ACT, with the f16 |se|^2 gathered alongside the
anchors riding in as the per-partition bias) run as single batched ops.
The perceptron GEMM (dist @ Wp, lag-2 behind the distance tiles) stays
float32r — full PE rate at free-dim 512, and dist ~ 32 is nearly
constant so Wp/dist rounding would bias whole output columns.

The AllGather is SPLIT INTO TWO HALF-ANCHOR MESHES (fp8 seA, -2x
scaled, plus an f16 |se|^2 row packed into two fp8 rows, 256 anchors
each): the distance loop runs all (rank, tile 0-1) pairs as soon as
mesh 1 lands, and mesh 2's transfer plus its reloads hide entirely
under that PE work (~26us), reaching the second-half tiles just in
time.  Reload descriptors are issued seA-then-s2 rank-ascending per
half, because the sync sequencer's blocking descriptor issue only
unblocks at each mesh's end and its issue order IS the
data-availability order.

Host-side packing: W1/W2/eT/Wp are pre-laid-out as [128, k-major]
arrays so SBUF tiles load with few large DMA descriptors (descriptor
issue is ~650ns each, serial on the sync sequencer); a "hot block"
(anchor columns + W1's first column block) heads the queue so the first
matmul fires ~6us after the sequencer boot barrier.
"""
import numpy as np
import ml_dtypes
from concourse import bass, bacc, tile, mybir, bass_utils, masks
from concourse import dve_ops as _dvo
from concourse.dve_spec import Spec as _Spec, Src0, Src1, C0, C1, C2, Bin as _Bin
from concourse.dve_spec import lower as _dve_lower
from concourse.dve_uop import AluOp as _DAlu, DveOpSpec as _DveOpSpec


def _register_mish_tail():
    """out = (in0*in1) * ~1/(in0+imm2): the mish tail vm*q/(q+2) in ONE
    DVE pass (BITWISE_NOT reciprocal seed + one Newton step, ~0.2% rel).
    Registered as a new custom-DVE op row; shas computed at import."""
    name = "MISH_TAIL_ANT"
    if name in _dvo._SUB_OPCODE_FOR_NAME:
        return next(o for o in _dvo.OPS if o.name == name)
    x = Src0 + C2
    nx = _Bin(_DAlu.BITWISE_NOT, x, x)
    y0 = nx * C0
    y1 = y0 * (C1 - x * y0)

    def _ref(in0, in1, c0, c1, c2):
        xx = in0.astype(np.float32) + np.float32(c2)
        nxx = (~xx.view(np.int32)).view(np.float32)
        yy0 = nxx * np.float32(c0)
        yy1 = yy0 * (np.float32(c1) - xx * yy0)
        return (in0.astype(np.float32) * in1.astype(np.float32)) * yy1

    spec = _Spec(body=(Src0 * Src1) * y1, reference=_ref)
    row = max(_dvo._SUB_OPCODE_FOR_NAME.values()) + 1
    shas = {}
    for ver in ("v3", "v4"):
        u = _dve_lower(spec, ver=ver)
        shas[ver] = _DveOpSpec(name=name, opcode=row, uops=u,
                               rd1_en=True).sha(ver)
    op = _dvo.DveOp(name, spec, subdim=False, uops_sha=shas,
                    perf_en={"v3": True, "v4": True})
    _dvo.OPS.append(op)
    _dvo.CUSTOM_DVE_SPECS[name] = spec
    _dvo._SUB_OPCODE_FOR_NAME[name] = row
    return op


MISH_TAIL = _register_mish_tail()
RECIP_C0, RECIP_C1 = -0.23549792, 2.0017324


def _patched_tables(arch):
    """Subset the ACT table sets (keeping dict order — act_func_set_id is
    positional) so Exp/Ln resolve only to natural_log_exp_and_others and
    Tanh only to exp_and_others. The default first-match choice alternates
    exp_and_others <-> natural_log on every exp/ln pair, paying a 1.3us
    table load each time."""
    from concourse.hw_specs import get_activation_tables as orig
    AFt = mybir.ActivationFunctionType
    out = {}
    for name, s in orig(arch).items():
        s = set(s)
        if name != "natural_log_exp_and_others":
            s.discard(AFt.Exp)
            s.discard(AFt.Ln)
            s.discard(AFt.Copy)
            s.discard(AFt.Identity)
        if name != "exp_and_others":
            s.discard(AFt.Tanh)
        out[name] = s
    return out

N, M, D, C = 8192, 4096, 512, 100
NCORES = 8
RPC = N // NCORES      # 1024 x-rows per core
MPC = M // NCORES      # 512 anchors encoded per core
KD = D // 128          # 4 contraction chunks of 128
NMT = M // 128         # 32 anchor tiles in the distance matmul
NRC = RPC // 512       # 2 row-chunks of 512
AGR = D + 2            # payload rows per rank: 512 seA + 2 (f16 s2)

F32 = mybir.dt.float32
F32R = mybir.dt.float32r
F16 = mybir.dt.float16
BF16 = mybir.dt.bfloat16
F8 = mybir.dt.float8e4
AF = mybir.ActivationFunctionType
ALU = mybir.AluOpType
DR = mybir.MatmulPerfMode.DoubleRow


def build_kernel():
    bacc.get_activation_tables = _patched_tables
    nc = bacc.Bacc("TRN2", target_bir_lowering=False, debug=False,
                   num_devices=NCORES)

    hot = nc.dram_tensor("hot", [128, KD * MPC + KD * 128], BF16,
                         kind="ExternalInput")
    eX = nc.dram_tensor("eX", [128, KD * RPC], BF16, kind="ExternalInput")
    W1p = nc.dram_tensor("W1p", [128, KD * D], BF16, kind="ExternalInput")
    W2p = nc.dram_tensor("W2p", [128, KD * D], BF16, kind="ExternalInput")
    bc = nc.dram_tensor("bc", [128, 3 * KD], F32, kind="ExternalInput")
    Wpp = nc.dram_tensor("Wpp", [128, NMT * C], F32, kind="ExternalInput")
    bp = nc.dram_tensor("bp", [1, C], F32, kind="ExternalInput")
    out = nc.dram_tensor("out", [RPC, C], F32, kind="ExternalOutput")

    with tile.TileContext(nc) as tc:
        _body(tc, hot, eX, W1p, W2p, bc, Wpp, bp, out)

    nc.compile()
    return nc


def _body(tc, hot, eX, W1p, W2p, bc, Wpp, bp, out):
    nc = tc.nc
    with (
        tc.tile_pool(name="const", bufs=1) as const,
        tc.tile_pool(name="wpool", bufs=1) as wpool,
        tc.tile_pool(name="spool", bufs=1) as spool,
        tc.tile_pool(name="xpool", bufs=1) as xpool,
        tc.tile_pool(name="gpool", bufs=1) as gpool,
        tc.tile_pool(name="mpool", bufs=2) as mpool,
        tc.tile_pool(name="dpool", bufs=8) as dpool,
        tc.tile_pool(name="zpool", bufs=2) as zpool,
        tc.tile_pool(name="ps", bufs=1, space="PSUM") as ps,
        tc.tile_pool(name="psz", bufs=1, space="PSUM") as psz,
        tc.tile_pool(name="dram", bufs=1, space="DRAM") as dram,
    ):
        # ---- need-ordered input loads; anchor-encode inputs head the
        # descriptor queue so the anchor->AllGather chain starts ASAP ----
        bc_sb = wpool.tile([128, 3 * KD], F32)
        nc.sync.dma_start(bc_sb[:], bc[:])
        b1c = bc_sb[:, 0:KD]
        b2c = bc_sb[:, KD:2 * KD]
        m2b2c = bc_sb[:, 2 * KD:3 * KD]
        eS_sb = spool.tile([128, KD, MPC], BF16)
        W1_sb = wpool.tile([128, KD, D], BF16)
        HB = KD * MPC + KD * 128
        nc.sync.dma_start(eS_sb[:, 0:2, :], hot[:, 0:2 * MPC])
        nc.sync.dma_start(eS_sb[:, 2:4, :], hot[:, 2 * MPC:4 * MPC])
        nc.sync.dma_start(
            W1_sb[:, :, 0:128],
            hot[:, KD * MPC:HB].rearrange("p (k f) -> p k f", k=KD))
        for k in range(KD):
            nc.sync.dma_start(W1_sb[:, k, 128:D],
                              W1p[:, D * k + 128:D * (k + 1)])
        W2_sb = wpool.tile([128, KD, D], BF16)
        eX_sb = xpool.tile([128, KD, RPC], BF16)
        for h in range(2):
            nc.sync.dma_start(W2_sb[:, 2 * h:2 * h + 2, :],
                              W2p[:, 2 * D * h:2 * D * (h + 1)])
        for h in range(2):
            nc.sync.dma_start(eX_sb[:, 2 * h:2 * h + 2, :],
                              eX[:, 2 * RPC * h:2 * RPC * (h + 1)])
        Wp_sb = wpool.tile([128, NMT, C], F32R)
        for h in range(2):
            nc.sync.dma_start(
                Wp_sb[:, h * (NMT // 2):(h + 1) * (NMT // 2), :],
                Wpp[:, h * NMT * C // 2:(h + 1) * NMT * C // 2].bitcast(F32R))
        bp_sb = wpool.tile([1, C], F32R)
        nc.sync.dma_start(bp_sb[:], bp[:].bitcast(F32R))

        # ---- constants (Copy resolves to exp_and_others, the same table
        # the encoder Exp uses, so startup costs one table load) ----
        ident = const.tile([C, C], F32)
        masks.make_identity(nc, ident[:])
        ones_f32 = const.tile([128, 1], F32)
        nc.vector.memset(ones_f32[:], 1.0)
        ones_col = const.tile([128, 1], BF16)    # lhsT for row-sum matmuls
        nc.scalar.activation(ones_col[:], ones_f32[:], AF.Copy)
        onesr_f32 = const.tile([1, 512], F32)
        nc.vector.memset(onesr_f32[:], 1.0)
        ones512 = const.tile([1, 512], F32R)     # rhs/lhsT for rank-1 terms
        nc.scalar.activation(ones512[:], onesr_f32[:], AF.Copy)

        def enc_head(u, vm, Wsb, bcol, abcol, src, w, vscale=1.0,
                     act_vst=3):
            """One 512-col chunk of a layer phase: matmuls + Exp (ACT) +
            v-stage.  act_vst = how many of the KD tiles put the v-stage on
            ACT (Identity with pre-scaled bias) vs DVE: x-phases use 3 for
            throughput balance; the latency-bound anchor phases use 0 so
            the DVE v-stages fully overlap the serial ACT exp chain."""
            ssl = slice(512 * w, 512 * (w + 1))
            for f in range(KD):
                vpt = ps.tile([128, 1024], F32, tag="mm", bufs=3)
                vps = vpt[:, 0:512]
                for k in range(KD):
                    nc.tensor.matmul(vps,
                                     Wsb[:, k, 128 * f:128 * (f + 1)],
                                     src[:, k, ssl],
                                     start=(k == 0), stop=(k == KD - 1))
                nc.scalar.activation(u[:, f, ssl], vps, AF.Exp,
                                     bias=bcol[:, f:f + 1])
                if f < act_vst:   # v-stage on ACT
                    nc.scalar.activation(vm[:, f, ssl], vps, AF.Identity,
                                         bias=abcol[:, f:f + 1],
                                         scale=vscale)
                elif vscale == 1.0:
                    nc.vector.tensor_scalar_add(vm[:, f, ssl], vps,
                                                bcol[:, f:f + 1])
                else:
                    nc.vector.tensor_scalar(vm[:, f, ssl], vps,
                                            bcol[:, f:f + 1], vscale,
                                            op0=ALU.add, op1=ALU.mult)

        def enc_tail(dst, u, vm, width):
            """mish tail for a whole phase: batched q = (w+2)w (second
            512-chunk offloaded to the otherwise-idle GPSIMD) and the fused
            MISH_TAIL custom op on DVE."""
            q = mpool.tile([128, KD, width], BF16, tag="q")
            nc.vector.scalar_tensor_tensor(q[:, :, :], u[:, :, :], 2.0,
                                           u[:, :, :],
                                           op0=ALU.add, op1=ALU.mult)
            nc.vector._custom_dve(
                MISH_TAIL,
                out=dst[:, :, :].rearrange("p k m -> p (k m)"),
                in0=q[:, :, :].rearrange("p k m -> p (k m)"),
                in1=vm[:, :, :].rearrange("p k m -> p (k m)"),
                s0=RECIP_C0, s1=RECIP_C1, imm2=2.0)

        # ---- encoder, software-pipelined: the anchor chain (aL1, aL2,
        # squares, collective) is first in every engine queue; x-phase
        # chunks are woven into the anchor chain's latency bubbles (aL1's
        # DVE tail hides xL1-w0's ACT work, etc).  LUT-free mish means no
        # table switches, so the interleave costs nothing on ACT. ----
        h_se = spool.tile([128, KD, MPC], BF16)
        seA_bf = spool.tile([128, KD, MPC], BF16)
        h_xe = xpool.tile([128, KD, RPC], BF16)
        xe_bf = xpool.tile([128, KD, RPC], BF16)
        uA = mpool.tile([128, KD, MPC], BF16, tag="ua")
        vA = mpool.tile([128, KD, MPC], BF16, tag="va")
        uX = mpool.tile([128, KD, RPC], BF16, tag="ux")
        vX = mpool.tile([128, KD, RPC], BF16, tag="vx")

        enc_head(uA, vA, W1_sb, b1c, b1c, eS_sb, 0, act_vst=0)  # aL1
        enc_tail(h_se, uA, vA, MPC)
        enc_head(uX, vX, W1_sb, b1c, b1c, eX_sb, 0)           # xL1 w0
        uA2 = mpool.tile([128, KD, MPC], BF16, tag="ua")
        vA2 = mpool.tile([128, KD, MPC], BF16, tag="va")
        enc_head(uA2, vA2, W2_sb, b2c, m2b2c, h_se, 0, vscale=-2.0,
                 act_vst=0)  # aL2
        enc_tail(seA_bf, uA2, vA2, MPC)

        # anchor tail: fp8 convert + s2 + the one collective
        seA8 = spool.tile([128, KD, MPC], F8)
        nc.scalar.activation(seA8[:, :, :], seA_bf[:, :, :], AF.Copy)
        sq_se = spool.tile([128, KD, MPC], BF16, tag="h_se")
        nc.vector.tensor_tensor(sq_se[:, :, :], seA_bf[:, :, :],
                                seA_bf[:, :, :], op=ALU.mult)
        s2pt = ps.tile([128, 1024], F32, tag="mm", bufs=3)
        for k in range(KD):
            nc.tensor.matmul(s2pt[0:1, 0:512], ones_col[:], sq_se[:, k, :],
                             start=(k == 0), stop=(k == KD - 1))
        s2row_sb = spool.tile([1, MPC], F16)
        nc.vector.tensor_scalar_mul(s2row_sb[:], s2pt[0:1, 0:512], 0.25)
        # two half-anchor AllGathers: the distance loop starts on the
        # (rank, tile 0-1) pairs as soon as mesh 1 lands; mesh 2 runs under
        # that PE work.  Payload per mesh: [512 seA rows + 2 f16-s2 rows,
        # 256 anchors] fp8.
        HM = MPC // 2
        ag_ins, ag_outs = [], []
        for h in range(2):
            agi = dram.tile([AGR, HM], F8, name=f"agi{h}")
            ago = dram.tile([NCORES * AGR, HM], F8, addr_space="Shared",
                            name=f"ago{h}")
            nc.sync.dma_start(
                agi[0:D, :].rearrange("(k p) m -> p k m", p=128),
                seA8[:, :, HM * h:HM * (h + 1)])
            nc.sync.dma_start(
                agi[D:D + 2, :].rearrange("(o a) b -> o (a b)", o=1),
                s2row_sb[:, HM * h:HM * (h + 1)].bitcast(F8))
            nc.gpsimd.collective_compute(
                "AllGather", ALU.bypass,
                replica_groups=[list(range(NCORES))],
                ins=[agi.opt()], outs=[ago.opt()])
            ag_ins.append(agi)
            ag_outs.append(ago)

        # ---- rest of the x columns (overlap the AllGather) ----
        enc_head(uX, vX, W1_sb, b1c, b1c, eX_sb, 1)           # xL1 w1
        enc_tail(h_xe, uX, vX, RPC)
        uX2 = mpool.tile([128, KD, RPC], BF16, tag="ux")
        vX2 = mpool.tile([128, KD, RPC], BF16, tag="vx")
        enc_head(uX2, vX2, W2_sb, b2c, b2c, h_xe, 0)          # xL2
        enc_head(uX2, vX2, W2_sb, b2c, b2c, h_xe, 1)
        enc_tail(xe_bf, uX2, vX2, RPC)
        xe8 = xpool.tile([128, KD, RPC], F8)
        for rc in range(NRC):
            csl = slice(512 * rc, 512 * (rc + 1))
            nc.scalar.activation(xe8[:, :, csl], xe_bf[:, :, csl], AF.Copy)

        # x2 broadcast tile: x2b[p, r] = |xe_r|^2 for every partition
        sq_xe = xpool.tile([128, KD, RPC], BF16, tag="h_xe")
        nc.vector.tensor_tensor(sq_xe[:, :, :], xe_bf[:, :, :],
                                xe_bf[:, :, :], op=ALU.mult)
        x2row_sb = xpool.tile([1, RPC], F32R)
        x2b_sb = xpool.tile([128, RPC], F32)
        xpt = ps.tile([128, 1024], F32, tag="mm", bufs=3)
        for rc in range(NRC):
            for k in range(KD):
                nc.tensor.matmul(xpt[0:1, 512 * rc:512 * (rc + 1)], ones_col[:],
                                 sq_xe[:, k, 512 * rc:512 * (rc + 1)],
                                 start=(k == 0), stop=(k == KD - 1),
                                 skip_group_check=True)
        nc.vector.tensor_copy(x2row_sb[:, :], xpt[0:1, :])
        xbt = ps.tile([128, 1024], F32, tag="mm", bufs=3)
        for rc in range(NRC):
            nc.tensor.matmul(xbt[:, 512 * rc:512 * (rc + 1)], ones512[:, :128],
                             x2row_sb[:, 512 * rc:512 * (rc + 1)],
                             start=True, stop=True, skip_group_check=True)
        nc.vector.tensor_copy(x2b_sb[:, :], xbt[:, :])

        # ---- load gathered anchors, rank-ascending (rank 0 gates the
        # first distance tile); two descriptors per rank, seA before s2.
        # The sync sequencer's blocking issue only unblocks at mesh-end,
        # so the issue order IS the availability order. ----
        seAg = [gpool.tile([128, KD, MPC], F8, name=f"seAg{g}")
                for g in range(NCORES)]
        s2all = gpool.tile([128, NCORES, KD], F16)
        for h in range(2):
            ago = ag_outs[h]
            for g in range(NCORES):
                if h == 0:
                    # mesh-1 reloads race the distance loop's consumption:
                    # two queue-parallel half-descriptors per rank
                    for kh in range(2):
                        nc.sync.dma_start(
                            seAg[g][:, 2 * kh:2 * kh + 2, HM * h:HM * (h + 1)],
                            ago[AGR * g + 256 * kh:AGR * g + 256 * (kh + 1),
                                :].rearrange("(k p) m -> p k m", p=128))
                else:
                    nc.sync.dma_start(
                        seAg[g][:, :, HM * h:HM * (h + 1)],
                        ago[AGR * g:AGR * g + D, :].rearrange(
                            "(k p) m -> p k m", p=128))
                rows = ago[AGR * g + D:AGR * g + D + 2, :].bitcast(F16)
                nc.sync.dma_start(
                    s2all[:, g, 2 * h:2 * h + 2],
                    rows.rearrange("a (f p) -> p (a f)", f=1, p=128))

        # ---- main fused loop: each anchor tile t pairs BOTH row-chunks in
        # one [128,1024] psum group — the x2 add and the sqrt run as single
        # batched ops (x2b is exactly the contiguous free-dim operand, and
        # both halves share the per-partition |se|^2 sqrt bias). DoubleRow
        # fp8: 2 PE passes per 512-deep half. Perceptron lags one tile. ----
        zt_ps = [psz.tile([C, 512], F32, name=f"ztps{rc}") for rc in range(NRC)]
        zpre_sb = zpool.tile([128, 2 * NRC * 2, C], BF16, bufs=1)
        zt_sbs = []
        for rc in range(NRC):
            nc.tensor.matmul(zt_ps[rc][:], bp_sb[:], ones512[:],
                             start=True, stop=False, skip_group_check=True)
        dist_tiles = {}
        torder = [4 * g + tl for h in range(2)
                  for g in range(NCORES) for tl in (2 * h, 2 * h + 1)]
        for ti in range(NMT):
            t = torder[ti]
            g, tl = t // (MPC // 128), t % (MPC // 128)
            d2ps = ps.tile([128, 1024], F32, tag="mm", bufs=3)
            for rc in range(NRC):
                for kp in range(2):
                    nc.tensor.matmul(d2ps[:, 512 * rc:512 * (rc + 1)],
                                     seAg[g][:, 2 * kp:2 * kp + 2,
                                             128 * tl:128 * (tl + 1)],
                                     xe8[:, 2 * kp:2 * kp + 2,
                                         512 * rc:512 * (rc + 1)],
                                     start=(kp == 0), stop=(kp == 1),
                                     perf_mode=DR, skip_group_check=True)
            nc.vector.tensor_tensor(d2ps[:, :], d2ps[:, :], x2b_sb[:, :],
                                    op=ALU.add)
            distT = dpool.tile([128, 1024], F32R, bufs=4)
            nc.scalar.activation(distT[:, :], d2ps[:, :], AF.Sqrt,
                                 bias=s2all[:, g, tl:tl + 1])
            dist_tiles[t] = distT
            if ti >= 2:
                tp_ = torder[ti - 2]
                dprev = dist_tiles.pop(tp_)
                for rc in range(NRC):
                    nc.tensor.matmul(zt_ps[rc][:], Wp_sb[:, tp_, :],
                                     dprev[:, 512 * rc:512 * (rc + 1)],
                                     start=False, stop=False,
                                     skip_group_check=True)
        for ti in (NMT - 2, NMT - 1):
            tp_ = torder[ti]
            dprev = dist_tiles.pop(tp_)
            for rc in range(NRC):
                nc.tensor.matmul(zt_ps[rc][:], Wp_sb[:, tp_, :],
                                 dprev[:, 512 * rc:512 * (rc + 1)],
                                 start=False, stop=(ti == NMT - 1),
                                 skip_group_check=True)
        for rc in range(NRC):
            # epilogue part A: psum copy + transposes
            zt_sb = zpool.tile([C, 512], F32, bufs=2, tag="zt")
            nc.vector.tensor_copy(zt_sb[:], zt_ps[rc][:])
            zt_sbs.append(zt_sb)
            for j in range(4):
                zrt = ps.tile([128, 1024], F32, tag="mm", bufs=3)
                nc.tensor.matmul(zrt[0:128, 0:C],
                                 zt_sb[:, 128 * j:128 * (j + 1)],
                                 ident[:], is_transpose=True)
                nc.vector.tensor_copy(zpre_sb[:, 4 * rc + j, :],
                                      zrt[0:128, 0:C])

        # ---- epilogue: one batched tanh + log-softmax pass (tanh output
        # is in [-1,1] so no max-subtraction is needed) ----
        NT = 2 * NRC * 2  # 8 tiles of 128 rows
        zth_sb = zpool.tile([128, NT, C], BF16, bufs=1)
        nc.scalar.activation(zth_sb[:, :, :], zpre_sb[:, :, :], AF.Tanh)
        e_sb = zpool.tile([128, NT, C], BF16, bufs=1, tag="zpre_sb")
        nc.scalar.activation(e_sb[:, :, :], zth_sb[:, :, :], AF.Exp)
        ssum = zpool.tile([128, NT], F32, bufs=1)
        nc.vector.tensor_reduce(ssum[:], e_sb[:, :, :],
                                axis=mybir.AxisListType.X, op=ALU.add)
        lns = zpool.tile([128, NT], F32, bufs=1)
        nc.scalar.activation(lns[:], ssum[:], AF.Ln)
        o_sb = zpool.tile([128, NT, C], F32, bufs=1, tag="osb")
        for jj in range(NT):
            nc.vector.tensor_scalar(o_sb[:, jj, :], zth_sb[:, jj, :],
                                    lns[:, jj:jj + 1], None,
                                    op0=ALU.subtract)
        for h in range(2):
            nc.sync.dma_start(
                out[512 * h:512 * (h + 1), :].rearrange(
                    "(j p) c -> p j c", p=128),
                o_sb[:, 4 * h:4 * (h + 1), :])


_NC_CACHE = None


def _get_nc():
    global _NC_CACHE
    if _NC_CACHE is None:
        _NC_CACHE = build_kernel()
    return _NC_CACHE


def make_in_maps(x, samples, W1, b1, W2, b2, Wp, bp):
    bf = ml_dtypes.bfloat16
    x = np.asarray(x, dtype=np.float32)
    samples = np.asarray(samples, dtype=np.float32)

    def kpack(w):  # [D, cols] -> [128, KD, cols] with [p, k, c] = w[128k+p, c]
        w = np.asarray(w, dtype=np.float32)
        return np.ascontiguousarray(
            w.reshape(KD, 128, -1).transpose(1, 0, 2).reshape(128, -1))

    W1b = kpack(W1).astype(bf)
    W2b = kpack(W2).astype(bf)
    Wpc = np.ascontiguousarray(
        np.asarray(Wp, dtype=np.float32).reshape(NMT, 128, C)
        .transpose(1, 0, 2).reshape(128, NMT * C))
    b1c = np.asarray(b1, dtype=np.float32).reshape(KD, 128).T
    b2c = np.asarray(b2, dtype=np.float32).reshape(KD, 128).T
    bcc = np.ascontiguousarray(np.concatenate([b1c, b2c, -2.0 * b2c], axis=1))
    bpc = np.ascontiguousarray(np.asarray(bp, dtype=np.float32).reshape(1, C))
    W1f0 = np.ascontiguousarray(
        W1b.reshape(128, KD, D)[:, :, 0:128].reshape(128, KD * 128))
    in_maps = []
    for g in range(NCORES):
        sT_g = kpack(samples[MPC * g:MPC * (g + 1), :].T).astype(bf)
        xT_g = kpack(x[RPC * g:RPC * (g + 1), :].T).astype(bf)
        hot = np.ascontiguousarray(np.concatenate([sT_g, W1f0], axis=1))
        in_maps.append({
            "hot": hot,
            "eX": np.ascontiguousarray(xT_g),
            "W1p": W1b, "W2p": W2b, "bc": bcc,
            "Wpp": Wpc, "bp": bpc,
        })
    return in_maps


def run(in_maps, trace=False):
    nc = _get_nc()
    res = bass_utils.run_bass_kernel_spmd(nc, in_maps,
                                          core_ids=list(range(NCORES)),
                                          trace=trace)
    outp = np.concatenate([res.results[g]["out"] for g in range(NCORES)],
                          axis=0).astype(np.float32)
    return outp, res


def kernel(x, samples, W1, b1, W2, b2, Wp, bp):
    in_maps = make_in_maps(x, samples, W1, b1, W2, b2, Wp, bp)
    outp, _ = run(in_maps, trace=False)
    return outp
